# revision 15
# baseline (speedup 1.0000x reference)
"""BatchMultiHeadGraphAttention TRN2 kernel.

Reference computation (per batch b, head h):
    h_prime = h[b] @ w[h]                          # [n, f]
    t = tanh(h_prime)
    src = t @ a_src[h];  dst = t @ a_dst[h]        # [n]
    s[i, j] = leaky_relu(src[i] + dst[j], 0.2)
    s = where(adj[b] | eye, s, -inf)
    attn = softmax(s, axis=-1)
    out[b, h] = attn @ h_prime + bias

Sharding: 8 cores, one (b, h) slab per core (bs=4 x H=2).

Key algebraic restructure vs the naive form: the pre-activation score is
rank-1 (s = src_i + dst_j), so
    exp(lrelu(s)) = max(exp(s), exp(0.2 s)) = v_j * max(u_i, p_i * r_j)
with u=exp(src), v=exp(dst), p=exp(0.2 src), r=exp(-0.8 dst).  The v_j
factor is folded into the value matrix (hp8 = fp8(v_j/4 * h_prime), with
a v_j/4 column computing Z), so the full n x n probability tensor is
built with just TWO elementwise passes (one stt max, one masked multiply)
and ZERO activation-table passes over n^2.  P is stored in fp8e4 and the
PV matmul runs in DoubleRow perf mode (0.5 PE cycles/row).

All n-indices on chip live in the permuted order sigma induced by the
u16-pair xbar transpose of adj (sigma(128*tau + p) = 256*(tau//2) + 2p +
tau%2), applied consistently to j (partitions) and i (free dim), so the
diagonal stays the diagonal and only the final output DMA un-permutes.
"""

import numpy as np

BS, N, H, F_IN, F_OUT = 4, 2048, 2, 768, 768
NCORES = 8

_CACHE = {}


def _build(has_bias: bool):
    import os
    import concourse.bass as bass
    import concourse.mybir as mybir
    import concourse.tile as tile
    from concourse import bacc
    from concourse.masks import make_identity

    dt = mybir.dt
    AF = mybir.ActivationFunctionType
    OP = mybir.AluOpType
    PM = mybir.MatmulPerfMode

    NT = N // 128            # 16 n-tiles (tau)
    KT = F_IN // 128         # 6 k-tiles
    NG = 2                   # i groups
    GW = N // NG             # 1024 group width
    CHG = GW // 128          # 8 i-chunks per group
    HPW = 772                # hp8 row stride (768 + Z col + pad)
    LN4 = float(np.log(4.0))

    nc = bacc.Bacc("TRN2", target_bir_lowering=False, debug=False,
                   num_devices=NCORES)

    d_h = nc.dram_tensor("h", [N, F_IN], dt.float32, kind="ExternalInput")
    d_adj = nc.dram_tensor("adj", [N, N], dt.uint8, kind="ExternalInput")
    d_w = nc.dram_tensor("w", [F_IN, F_OUT], dt.float32, kind="ExternalInput")
    d_asrc = nc.dram_tensor("a_src", [F_OUT], dt.float32, kind="ExternalInput")
    d_adst = nc.dram_tensor("a_dst", [F_OUT], dt.float32, kind="ExternalInput")
    if has_bias:
        d_bias = nc.dram_tensor("bias", [F_OUT], dt.float32,
                                kind="ExternalInput")
    d_out = nc.dram_tensor("out", [N, F_OUT], dt.float32,
                           kind="ExternalOutput")

    def sig_off(tau):
        # on-chip position m = 128*tau + p  <->  logical n index
        # sigma(m) = 256*(tau//2) + 2*p + (tau % 2)
        return 256 * (tau // 2) + (tau % 2)

    with tile.TileContext(nc) as tc:
        with tc.tile_pool(name="const", bufs=1) as cpool, \
             tc.tile_pool(name="persist", bufs=1) as pp:
            # ---- constants ----
            ident = cpool.tile([128, 128], dt.float32, tag="ident")
            make_identity(nc, ident[:])
            eye_u8 = cpool.tile([128, 128], dt.uint8, tag="eye_u8")
            nc.vector.tensor_copy(eye_u8[:], ident[:])
            c_nln4 = cpool.tile([128, 1], dt.float32, tag="c_nln4")
            nc.gpsimd.memset(c_nln4[:], -LN4)

            if has_bias:
                bias_row = cpool.tile([1, F_OUT], dt.float32, tag="bias_row")
                nc.sync.dma_start(bias_row[:],
                                  d_bias.ap().rearrange("(o f) -> o f", o=1))
                bias_bc = pp.tile([128, F_OUT], dt.float32, tag="bias_bc")
                nc.gpsimd.partition_broadcast(bias_bc[:], bias_row[:])

            # ---- persistent buffers ----
            # hpb[j, tau*HPW + f] = bf16(v_j/4 * h_prime[j, f]); col 768 = v_j/4
            hp8 = pp.tile([128, NT * HPW], dt.bfloat16, tag="hp8")
            src_col = pp.tile([128, NT], dt.float32, tag="src_col")
            dst_col = pp.tile([128, NT], dt.float32, tag="dst_col")
            v4_col = pp.tile([128, NT], dt.float32, tag="v4_col")
            r_col = pp.tile([128, NT], dt.float32, tag="r_col")
            rz_col = pp.tile([128, NT], dt.float32, tag="rz_col")

            # adj staging: 8 tiles [128, N] u16 (xbar transpose of u16 pairs)
            stag = [pp.tile([128, N], dt.uint16, tag=f"stag{t}",
                            name=f"stag{t}")
                    for t in range(NT // 2)]
            adj16 = d_adj.ap().bitcast(dt.uint16)       # [N, N//2]

            # ---- phase 1: hT = transpose(h) via PE; load w ----
            with tc.tile_pool(name="ph1", bufs=6) as hpool, \
                 tc.tile_pool(name="ph1t", bufs=1) as htp, \
                 tc.tile_pool(name="ph1ps", bufs=2, space="PSUM") as psum1:
                # w as bf16 (DMA f32 then cast on gpsimd)
                wr = [htp.tile([128, F_OUT], dt.bfloat16, tag=f"wr{k}",
                               name=f"wr{k}")
                      for k in range(KT)]
                for k in range(KT):
                    wtmp = hpool.tile([128, F_OUT], dt.float32, tag="hin",
                                      name=f"wtmp{k}")
                    nc.sync.dma_start(wtmp[:], d_w[128 * k:128 * (k + 1), :])
                    nc.gpsimd.tensor_copy(wr[k][:], wtmp[:])
                # a_src/a_dst broadcast [128, F_OUT] (for the dots)
                asrc_row = htp.tile([1, F_OUT], dt.float32, tag="asrc_row")
                nc.sync.dma_start(asrc_row[:],
                                  d_asrc.ap().rearrange("(o f) -> o f", o=1))
                adst_row = htp.tile([1, F_OUT], dt.float32, tag="adst_row")
                nc.sync.dma_start(adst_row[:],
                                  d_adst.ap().rearrange("(o f) -> o f", o=1))
                asrc_bc = htp.tile([128, F_OUT], dt.float32, tag="asrc_bc")
                nc.gpsimd.partition_broadcast(asrc_bc[:], asrc_row[:])
                adst_bc = htp.tile([128, F_OUT], dt.float32, tag="adst_bc")
                nc.gpsimd.partition_broadcast(adst_bc[:], adst_row[:])

                hT = [htp.tile([128, N], dt.bfloat16, tag=f"hT{k}",
                               name=f"hT{k}")
                      for k in range(KT)]
                for ngrp in range(NT // 4):
                    hr = []
                    for nn in range(4):
                        t = 4 * ngrp + nn
                        ht_in = hpool.tile([128, F_IN], dt.float32, tag="hin")
                        nc.sync.dma_start(ht_in[:],
                                          d_h[128 * t:128 * (t + 1), :])
                        hr.append(ht_in)
                    for k in range(KT):
                        ps = psum1.tile([128, 512], dt.float32, tag="tps")
                        for nn in range(4):
                            nc.tensor.transpose(
                                ps[:, 128 * nn:128 * (nn + 1)],
                                hr[nn][:, 128 * k:128 * (k + 1)],
                                ident[:])
                        # store hT in sigma-permuted column order:
                        # in pos 256*t + 2*q + o -> out pos 256*t+128*o+q
                        psperm = ps[:].rearrange(
                            "p (t q o) -> p t o q", t=2, q=128, o=2)
                        # gpsimd cannot access PSUM; alternate DVE/ACT
                        if k % 2 == 0:
                            nc.vector.tensor_copy(
                                hT[k][:, 512 * ngrp:512 * (ngrp + 1)], psperm)
                        else:
                            nc.scalar.copy(
                                hT[k][:, 512 * ngrp:512 * (ngrp + 1)], psperm)

                # adj xbar transposes issued after the h/w loads so they
                # don't hog the HWDGE rings at kernel start
                for t in range(NT // 2):
                    nc.sync.dma_start(stag[t][:],
                                      adj16[:, 128 * t:128 * (t + 1)],
                                      transpose=True)
                # OR the identity into the adj staging at the 16 diagonal
                # blocks: within stag[tau//2], partition q, byte
                # 512*(tau//2) + 4*k + 3*(tau%2) holds adj[logical_i(k),
                # logical_j(q)] for the tau diag block; k==q is the diagonal.
                for tau in range(NT):
                    off = 512 * (tau // 2) + 3 * (tau % 2)
                    dview = stag[tau // 2][:].bitcast(dt.uint8)[
                        :, off:off + 509:4]
                    nc.vector.tensor_tensor(dview, dview, eye_u8[:],
                                            op=OP.max)

                # ---- phase 2: h_prime per tau; tanh; dots; fp8 evict ----
                with tc.tile_pool(name="ph2", bufs=3) as tpool, \
                     tc.tile_pool(name="ph2ps", bufs=3, space="PSUM") as psum2:
                    for tau in range(NT):
                        ps = psum2.tile([128, F_OUT], dt.float32, tag="hpps")
                        for k in range(KT):
                            lhsT = hT[k][:, 128 * tau:128 * (tau + 1)]
                            nc.tensor.matmul(ps[:, 0:512], lhsT,
                                             wr[k][:, 0:512],
                                             start=(k == 0), stop=(k == KT - 1))
                            nc.tensor.matmul(ps[:, 512:F_OUT], lhsT,
                                             wr[k][:, 512:F_OUT],
                                             start=(k == 0), stop=(k == KT - 1))
                        # unscaled fp8 evict (the v_j/4 scale is applied
                        # in-place later, off the psum critical path)
                        nc.scalar.activation(
                            hp8[:, HPW * tau:HPW * tau + F_OUT],
                            ps[:], AF.Copy)
                        # tanh + the two dots
                        tnh = tpool.tile([128, F_OUT], dt.float32, tag="tnh")
                        nc.scalar.activation(tnh[:], ps[:], AF.Tanh)
                        scr = tpool.tile([128, F_OUT], dt.float32, tag="scr")
                        nc.vector.tensor_tensor_reduce(
                            scr[:], tnh[:], asrc_bc[:], 1.0, 0.0,
                            op0=OP.mult, op1=OP.add,
                            accum_out=src_col[:, tau:tau + 1])
                        scr2 = tpool.tile([128, F_OUT], dt.float32, tag="scr")
                        nc.vector.tensor_tensor_reduce(
                            scr2[:], tnh[:], adst_bc[:], 1.0, 0.0,
                            op0=OP.mult, op1=OP.add,
                            accum_out=dst_col[:, tau:tau + 1])

            # ---- phase 3+4, per i-group g (= tau half-range) ----
            # group g's u/p factors need only src from taus [8g, 8g+8), so
            # g=0's P build starts while phase 2 finishes taus 8..15.
            HT = NT // NG  # taus per group
            u_bc = pp.tile([128, N], dt.float32, tag="u_bc")
            p_bc = pp.tile([128, N], dt.float32, tag="p_bc")
            hp8v = hp8[:].rearrange("p (t f) -> p t f", t=NT)
            with tc.tile_pool(name="ph3", bufs=2) as p3, \
                 tc.tile_pool(name="ph3ps", bufs=2, space="PSUM") as psum3, \
                 tc.tile_pool(name="pg", bufs=4) as sp, \
                 tc.tile_pool(name="pgP", bufs=1) as ppool, \
                 tc.tile_pool(name="pgo", bufs=4) as opool, \
                 tc.tile_pool(name="pgps", bufs=3, space="PSUM") as psum5:
                for g in range(NG):
                    t0 = HT * g
                    # per-half j-side factors: v/4 = exp(dst - ln4),
                    # r = exp(-0.8 dst); then scale hp8 rows by v/4 (Pool)
                    nc.scalar.activation(v4_col[:, t0:t0 + HT],
                                         dst_col[:, t0:t0 + HT],
                                         AF.Exp, bias=c_nln4[:])
                    nc.scalar.activation(r_col[:, t0:t0 + HT],
                                         dst_col[:, t0:t0 + HT],
                                         AF.Exp, scale=-0.8)
                    nc.vector.tensor_copy(hp8v[:, t0:t0 + HT, F_OUT],
                                          v4_col[:, t0:t0 + HT])
                    for tau in range(t0, t0 + HT):
                        nc.gpsimd.tensor_scalar(
                            hp8[:, HPW * tau:HPW * tau + F_OUT],
                            hp8[:, HPW * tau:HPW * tau + F_OUT],
                            v4_col[:, tau:tau + 1], None, op0=OP.mult)
                    # i-side factors for this group: u = exp(src),
                    # p = exp(0.2 src), broadcast along partitions
                    st = psum3.tile([HT, 128], dt.float32, tag="srcT")
                    nc.tensor.transpose(st[:], src_col[:, t0:t0 + HT],
                                        ident[:])
                    uT = p3.tile([HT, 128], dt.float32, tag="uT")
                    nc.scalar.activation(uT[:], st[:], AF.Exp)
                    pT = p3.tile([HT, 128], dt.float32, tag="pT")
                    nc.scalar.activation(pT[:], st[:], AF.Exp, scale=0.2)
                    u_row = p3.tile([1, GW], dt.float32, tag="u_row")
                    nc.sync.dma_start(
                        u_row[:].rearrange("o (t p) -> o t p", t=HT), uT[:])
                    p_row = p3.tile([1, GW], dt.float32, tag="p_row")
                    nc.sync.dma_start(
                        p_row[:].rearrange("o (t p) -> o t p", t=HT), pT[:])
                    nc.gpsimd.partition_broadcast(
                        u_bc[:, GW * g:GW * (g + 1)], u_row[:])
                    nc.gpsimd.partition_broadcast(
                        p_bc[:, GW * g:GW * (g + 1)], p_row[:])

                    Pg = ppool.tile([128, NT * GW], dt.bfloat16, tag=f"P{g}",
                                    name=f"P{g}")
                    for tau in range(NT):
                        # smax = max(u_i, p_i * r_j)
                        smax = sp.tile([128, GW], dt.float32, tag="smax")
                        nc.vector.scalar_tensor_tensor(
                            smax[:], p_bc[:, GW * g:GW * (g + 1)],
                            r_col[:, tau:tau + 1],
                            u_bc[:, GW * g:GW * (g + 1)],
                            op0=OP.mult, op1=OP.max)
                        # P = mask * smax (fp8), mask via the u8 adj view
                        u8v = stag[tau // 2][:].bitcast(dt.uint8).rearrange(
                            "p (b k hh o) -> p b hh k o",
                            b=N // 256, k=128, hh=2, o=2)
                        smax3 = smax[:].rearrange(
                            "p (b hh k) -> p b hh k", b=GW // 256, hh=2, k=128)
                        P3 = Pg[:, GW * tau:GW * (tau + 1)].rearrange(
                            "p (b hh k) -> p b hh k", b=GW // 256, hh=2, k=128)
                        # mask multiply: mostly on Pool (it only supports
                        # mult), the rest on DVE for balance
                        b2eng = nc.vector if (NT * g + tau) % 3 == 0 else \
                            nc.gpsimd
                        b2eng.tensor_tensor(
                            P3,
                            u8v[:, (GW // 256) * g:(GW // 256) * (g + 1),
                                :, :, tau % 2],
                            smax3, op=OP.mult)
                    # PV + Z on PE in bf16 (1 cyc/row)
                    PT = Pg[:].rearrange("p (t i) -> p t i", t=NT)
                    HR = hp8[:].rearrange("p (t f) -> p t f", t=NT)
                    for c in range(CHG):
                        ci = CHG * g + c
                        ps = psum5.tile([128, HPW], dt.float32, tag="pvps")
                        # Z column (with the 512:768 chunk) first so rz is
                        # ready before the 0:512 chunk finishes
                        for u in range(NT):
                            nc.tensor.matmul(
                                ps[:, 512:F_OUT + 1],
                                PT[:, u, 128 * c:128 * (c + 1)],
                                HR[:, u, 512:F_OUT + 1],
                                start=(u == 0), stop=(u == NT - 1))
                        nc.vector.reciprocal(rz_col[:, ci:ci + 1],
                                             ps[:, F_OUT:F_OUT + 1])
                        for u in range(NT):
                            nc.tensor.matmul(
                                ps[:, 0:512],
                                PT[:, u, 128 * c:128 * (c + 1)],
                                HR[:, u, 0:512],
                                start=(u == 0), stop=(u == NT - 1))
                        ob = opool.tile([128, F_OUT], dt.float32, tag="ob")
                        nc.scalar.activation(ob[:], ps[:, 0:F_OUT], AF.Copy,
                                             scale=rz_col[:, ci:ci + 1])
                        if has_bias:
                            nc.gpsimd.tensor_tensor(ob[:], ob[:], bias_bc[:],
                                                    op=OP.add)
                        base = sig_off(ci)
                        nc.sync.dma_start(d_out[base:base + 255:2, 0:F_OUT],
                                          ob[:])

    nc.compile()
    return nc


def _get_program(has_bias: bool):
    key = ("prog", has_bias)
    if key not in _CACHE:
        _CACHE[key] = _build(has_bias)
    return _CACHE[key]


def kernel(h, adj, w, a_src, a_dst, bias):
    from concourse.bass_utils import run_bass_kernel_spmd

    h = np.ascontiguousarray(np.asarray(h, dtype=np.float32))
    adj_u8 = np.ascontiguousarray(np.asarray(adj).astype(np.uint8))
    w = np.ascontiguousarray(np.asarray(w, dtype=np.float32))
    a_src = np.asarray(a_src, dtype=np.float32).reshape(H, F_OUT)
    a_dst = np.asarray(a_dst, dtype=np.float32).reshape(H, F_OUT)
    bias = np.asarray(bias, dtype=np.float32).reshape(F_OUT)
    has_bias = bool(np.any(bias))

    nc = _get_program(has_bias)

    in_maps = []
    for core in range(NCORES):
        b, hd = core // H, core % H
        m = {
            "h": h[b],
            "adj": adj_u8[b],
            "w": w[hd],
            "a_src": a_src[hd],
            "a_dst": a_dst[hd],
        }
        if has_bias:
            m["bias"] = bias
        in_maps.append(m)

    res = run_bass_kernel_spmd(nc, in_maps, list(range(NCORES)))
    out = np.empty((BS, H, N, F_OUT), dtype=np.float32)
    for core in range(NCORES):
        b, hd = core // H, core % H
        out[b, hd] = res.results[core]["out"]
    return out


# revision 17
# speedup vs baseline: 1.0418x; 1.0418x over previous
"""BatchMultiHeadGraphAttention TRN2 kernel.

Reference computation (per batch b, head h):
    h_prime = h[b] @ w[h]                          # [n, f]
    t = tanh(h_prime)
    src = t @ a_src[h];  dst = t @ a_dst[h]        # [n]
    s[i, j] = leaky_relu(src[i] + dst[j], 0.2)
    s = where(adj[b] | eye, s, -inf)
    attn = softmax(s, axis=-1)
    out[b, h] = attn @ h_prime + bias

Sharding: 8 cores, one (b, h) slab per core (bs=4 x H=2).

Key algebraic restructure vs the naive form: the pre-activation score is
rank-1 (s = src_i + dst_j), so
    exp(lrelu(s)) = max(exp(s), exp(0.2 s)) = v_j * max(u_i, p_i * r_j)
with u=exp(src), v=exp(dst), p=exp(0.2 src), r=exp(-0.8 dst).  The v_j
factor is folded into the value matrix (hp8 = fp8(v_j/4 * h_prime), with
a v_j/4 column computing Z), so the full n x n probability tensor is
built with just TWO elementwise passes (one stt max, one masked multiply)
and ZERO activation-table passes over n^2.  P is stored in fp8e4 and the
PV matmul runs in DoubleRow perf mode (0.5 PE cycles/row).

All n-indices on chip live in the permuted order sigma induced by the
u16-pair xbar transpose of adj (sigma(128*tau + p) = 256*(tau//2) + 2p +
tau%2), applied consistently to j (partitions) and i (free dim), so the
diagonal stays the diagonal and only the final output DMA un-permutes.
"""

import numpy as np

BS, N, H, F_IN, F_OUT = 4, 2048, 2, 768, 768
NCORES = 8

_CACHE = {}


def _build(has_bias: bool):
    import os
    import concourse.bass as bass
    import concourse.mybir as mybir
    import concourse.tile as tile
    from concourse import bacc
    from concourse.masks import make_identity

    dt = mybir.dt
    AF = mybir.ActivationFunctionType
    OP = mybir.AluOpType
    PM = mybir.MatmulPerfMode

    NT = N // 128            # 16 n-tiles (tau)
    KT = F_IN // 128         # 6 k-tiles
    NG = 2                   # i groups
    GW = N // NG             # 1024 group width
    CHG = GW // 128          # 8 i-chunks per group
    HPW = 772                # hp8 row stride (768 + Z col + pad)
    LN4 = float(np.log(4.0))

    nc = bacc.Bacc("TRN2", target_bir_lowering=False, debug=False,
                   num_devices=NCORES)

    d_h = nc.dram_tensor("h", [N, F_IN], dt.float32, kind="ExternalInput")
    d_adj = nc.dram_tensor("adj", [N, N], dt.uint8, kind="ExternalInput")
    d_w = nc.dram_tensor("w", [F_IN, F_OUT], dt.float32, kind="ExternalInput")
    d_asrc = nc.dram_tensor("a_src", [F_OUT], dt.float32, kind="ExternalInput")
    d_adst = nc.dram_tensor("a_dst", [F_OUT], dt.float32, kind="ExternalInput")
    if has_bias:
        d_bias = nc.dram_tensor("bias", [F_OUT], dt.float32,
                                kind="ExternalInput")
    d_out = nc.dram_tensor("out", [N, F_OUT], dt.float32,
                           kind="ExternalOutput")

    def sig_off(tau):
        # on-chip position m = 128*tau + p  <->  logical n index
        # sigma(m) = 256*(tau//2) + 2*p + (tau % 2)
        return 256 * (tau // 2) + (tau % 2)

    with tile.TileContext(nc) as tc:
        with tc.tile_pool(name="const", bufs=1) as cpool, \
             tc.tile_pool(name="persist", bufs=1) as pp:
            # ---- constants ----
            ident = cpool.tile([128, 128], dt.float32, tag="ident")
            make_identity(nc, ident[:])
            eye_u8 = cpool.tile([128, 128], dt.uint8, tag="eye_u8")
            nc.vector.tensor_copy(eye_u8[:], ident[:])
            c_nln4 = cpool.tile([128, 1], dt.float32, tag="c_nln4")
            nc.gpsimd.memset(c_nln4[:], -LN4)

            if has_bias:
                bias_row = cpool.tile([1, F_OUT], dt.float32, tag="bias_row")
                nc.sync.dma_start(bias_row[:],
                                  d_bias.ap().rearrange("(o f) -> o f", o=1))
                bias_bc = pp.tile([128, F_OUT], dt.float32, tag="bias_bc")
                nc.gpsimd.partition_broadcast(bias_bc[:], bias_row[:])

            # ---- persistent buffers ----
            # hpb[j, tau*HPW + f] = bf16(v_j/4 * h_prime[j, f]); col 768 = v_j/4
            hp8 = pp.tile([128, NT * HPW], dt.bfloat16, tag="hp8")
            src_col = pp.tile([128, NT], dt.float32, tag="src_col")
            dst_col = pp.tile([128, NT], dt.float32, tag="dst_col")
            v4_col = pp.tile([128, NT], dt.float32, tag="v4_col")
            r_col = pp.tile([128, NT], dt.float32, tag="r_col")
            rz_col = pp.tile([128, NT], dt.float32, tag="rz_col")

            hp8v = hp8[:].rearrange("p (t f) -> p t f", t=NT)

            # adj staging: 8 tiles [128, N] u16 (xbar transpose of u16 pairs)
            stag = [pp.tile([128, N], dt.uint16, tag=f"stag{t}",
                            name=f"stag{t}")
                    for t in range(NT // 2)]
            adj16 = d_adj.ap().bitcast(dt.uint16)       # [N, N//2]

            # ---- phase 1: hT = transpose(h) via PE; load w ----
            with tc.tile_pool(name="ph1", bufs=6) as hpool, \
                 tc.tile_pool(name="ph1t", bufs=1) as htp, \
                 tc.tile_pool(name="ph1ps", bufs=2, space="PSUM") as psum1:
                # w as bf16 (DMA f32 then cast on gpsimd)
                wr = [htp.tile([128, F_OUT], dt.bfloat16, tag=f"wr{k}",
                               name=f"wr{k}")
                      for k in range(KT)]
                for k in range(KT):
                    wtmp = hpool.tile([128, F_OUT], dt.float32, tag="hin",
                                      name=f"wtmp{k}")
                    nc.sync.dma_start(wtmp[:], d_w[128 * k:128 * (k + 1), :])
                    nc.gpsimd.tensor_copy(wr[k][:], wtmp[:])
                # a_src/a_dst broadcast [128, F_OUT] (for the dots)
                asrc_row = htp.tile([1, F_OUT], dt.float32, tag="asrc_row")
                nc.sync.dma_start(asrc_row[:],
                                  d_asrc.ap().rearrange("(o f) -> o f", o=1))
                adst_row = htp.tile([1, F_OUT], dt.float32, tag="adst_row")
                nc.sync.dma_start(adst_row[:],
                                  d_adst.ap().rearrange("(o f) -> o f", o=1))
                asrc_bc = htp.tile([128, F_OUT], dt.float32, tag="asrc_bc")
                nc.gpsimd.partition_broadcast(asrc_bc[:], asrc_row[:])
                adst_bc = htp.tile([128, F_OUT], dt.float32, tag="adst_bc")
                nc.gpsimd.partition_broadcast(adst_bc[:], adst_row[:])

                hT = [htp.tile([128, N], dt.bfloat16, tag=f"hT{k}",
                               name=f"hT{k}")
                      for k in range(KT)]
                for ngrp in range(NT // 4):
                    hr = []
                    for nn in range(4):
                        t = 4 * ngrp + nn
                        ht_in = hpool.tile([128, F_IN], dt.float32, tag="hin")
                        nc.sync.dma_start(ht_in[:],
                                          d_h[128 * t:128 * (t + 1), :])
                        hr.append(ht_in)
                    for k in range(KT):
                        ps = psum1.tile([128, 512], dt.float32, tag="tps")
                        for nn in range(4):
                            nc.tensor.transpose(
                                ps[:, 128 * nn:128 * (nn + 1)],
                                hr[nn][:, 128 * k:128 * (k + 1)],
                                ident[:])
                        # store hT in sigma-permuted column order:
                        # in pos 256*t + 2*q + o -> out pos 256*t+128*o+q
                        psperm = ps[:].rearrange(
                            "p (t q o) -> p t o q", t=2, q=128, o=2)
                        # gpsimd cannot access PSUM; alternate DVE/ACT
                        if k % 2 == 0:
                            nc.vector.tensor_copy(
                                hT[k][:, 512 * ngrp:512 * (ngrp + 1)], psperm)
                        else:
                            nc.scalar.copy(
                                hT[k][:, 512 * ngrp:512 * (ngrp + 1)], psperm)

                # adj xbar transposes issued after the h/w loads so they
                # don't hog the HWDGE rings at kernel start
                for t in range(NT // 2):
                    nc.sync.dma_start(stag[t][:],
                                      adj16[:, 128 * t:128 * (t + 1)],
                                      transpose=True)
                # OR the identity into the adj staging at the 16 diagonal
                # blocks: within stag[tau//2], partition q, byte
                # 512*(tau//2) + 4*k + 3*(tau%2) holds adj[logical_i(k),
                # logical_j(q)] for the tau diag block; k==q is the diagonal.
                for tau in range(NT):
                    off = 512 * (tau // 2) + 3 * (tau % 2)
                    dview = stag[tau // 2][:].bitcast(dt.uint8)[
                        :, off:off + 509:4]
                    nc.vector.tensor_tensor(dview, dview, eye_u8[:],
                                            op=OP.max)

                # ---- phase 2: h_prime per tau; tanh; dots; fp8 evict ----
                with tc.tile_pool(name="ph2", bufs=3) as tpool, \
                     tc.tile_pool(name="ph2ps", bufs=2, space="PSUM") as psum2:
                    for tau in range(NT):
                        ps = psum2.tile([128, F_OUT], dt.float32, tag="hpps")
                        for k in range(KT):
                            lhsT = hT[k][:, 128 * tau:128 * (tau + 1)]
                            nc.tensor.matmul(ps[:, 0:512], lhsT,
                                             wr[k][:, 0:512],
                                             start=(k == 0), stop=(k == KT - 1))
                            nc.tensor.matmul(ps[:, 512:F_OUT], lhsT,
                                             wr[k][:, 512:F_OUT],
                                             start=(k == 0), stop=(k == KT - 1))
                        # unscaled fp8 evict (the v_j/4 scale is applied
                        # in-place later, off the psum critical path)
                        nc.scalar.activation(
                            hp8[:, HPW * tau:HPW * tau + F_OUT],
                            ps[:], AF.Copy)
                        # tanh + the two dots
                        tnh = tpool.tile([128, F_OUT], dt.float32, tag="tnh")
                        nc.scalar.activation(tnh[:], ps[:], AF.Tanh)
                        scr = tpool.tile([128, F_OUT], dt.float32, tag="scr")
                        nc.vector.tensor_tensor_reduce(
                            scr[:], tnh[:], asrc_bc[:], 1.0, 0.0,
                            op0=OP.mult, op1=OP.add,
                            accum_out=src_col[:, tau:tau + 1])
                        scr2 = tpool.tile([128, F_OUT], dt.float32, tag="scr")
                        nc.vector.tensor_tensor_reduce(
                            scr2[:], tnh[:], adst_bc[:], 1.0, 0.0,
                            op0=OP.mult, op1=OP.add,
                            accum_out=dst_col[:, tau:tau + 1])
                        if tau % 4 == 3:
                            # batched j-side factors for taus [tau-3, tau]:
                            # v/4 = exp(dst - ln4), r = exp(-0.8 dst), then
                            # scale hpb rows by v/4 (Pool) + write Z cols
                            q0 = tau - 3
                            nc.scalar.activation(v4_col[:, q0:tau + 1],
                                                 dst_col[:, q0:tau + 1],
                                                 AF.Exp, bias=c_nln4[:])
                            nc.scalar.activation(r_col[:, q0:tau + 1],
                                                 dst_col[:, q0:tau + 1],
                                                 AF.Exp, scale=-0.8)
                            nc.vector.tensor_copy(
                                hp8v[:, q0:tau + 1, F_OUT],
                                v4_col[:, q0:tau + 1])
                            for q in range(q0, tau + 1):
                                nc.gpsimd.tensor_scalar(
                                    hp8[:, HPW * q:HPW * q + F_OUT],
                                    hp8[:, HPW * q:HPW * q + F_OUT],
                                    v4_col[:, q:q + 1], None, op0=OP.mult)

            # ---- phase 3+4, per i-group g (= tau half-range) ----
            # group g's u/p factors need only src from taus [8g, 8g+8), so
            # g=0's P build starts while phase 2 finishes taus 8..15.
            HT = NT // NG  # taus per group
            u_bc = pp.tile([128, N], dt.float32, tag="u_bc")
            p_bc = pp.tile([128, N], dt.float32, tag="p_bc")
            with tc.tile_pool(name="ph3", bufs=2) as p3, \
                 tc.tile_pool(name="pg", bufs=4) as sp, \
                 tc.tile_pool(name="pgP", bufs=1) as ppool, \
                 tc.tile_pool(name="pgo", bufs=4) as opool, \
                 tc.tile_pool(name="pgps", bufs=4, space="PSUM") as psum5:
                for g in range(NG):
                    t0 = HT * g
                    # i-side factors for this group: exp then gather-DMA
                    # (per-element descriptors) into a [1, GW] row, then
                    # partition-broadcast
                    ux = p3.tile([128, HT], dt.float32, tag="ux")
                    nc.scalar.activation(ux[:], src_col[:, t0:t0 + HT],
                                         AF.Exp)
                    px = p3.tile([128, HT], dt.float32, tag="px")
                    nc.scalar.activation(px[:], src_col[:, t0:t0 + HT],
                                         AF.Exp, scale=0.2)
                    u_row = p3.tile([1, GW], dt.float32, tag="u_row")
                    nc.sync.dma_start(
                        u_row[:].rearrange("o (t p) -> o t p", t=HT),
                        ux[:].rearrange("p t -> t p"))
                    p_row = p3.tile([1, GW], dt.float32, tag="p_row")
                    nc.sync.dma_start(
                        p_row[:].rearrange("o (t p) -> o t p", t=HT),
                        px[:].rearrange("p t -> t p"))
                    nc.gpsimd.partition_broadcast(
                        u_bc[:, GW * g:GW * (g + 1)], u_row[:])
                    nc.gpsimd.partition_broadcast(
                        p_bc[:, GW * g:GW * (g + 1)], p_row[:])

                    Pg = ppool.tile([128, NT * GW], dt.bfloat16, tag=f"P{g}",
                                    name=f"P{g}")
                    for tau in range(NT):
                        # smax = max(u_i, p_i * r_j)
                        smax = sp.tile([128, GW], dt.float32, tag="smax")
                        nc.vector.scalar_tensor_tensor(
                            smax[:], p_bc[:, GW * g:GW * (g + 1)],
                            r_col[:, tau:tau + 1],
                            u_bc[:, GW * g:GW * (g + 1)],
                            op0=OP.mult, op1=OP.max)
                        # P = mask * smax (fp8), mask via the u8 adj view
                        u8v = stag[tau // 2][:].bitcast(dt.uint8).rearrange(
                            "p (b k hh o) -> p b hh k o",
                            b=N // 256, k=128, hh=2, o=2)
                        smax3 = smax[:].rearrange(
                            "p (b hh k) -> p b hh k", b=GW // 256, hh=2, k=128)
                        P3 = Pg[:, GW * tau:GW * (tau + 1)].rearrange(
                            "p (b hh k) -> p b hh k", b=GW // 256, hh=2, k=128)
                        # mask multiply: mostly on Pool (it only supports
                        # mult), the rest on DVE for balance
                        b2eng = nc.vector if (NT * g + tau) % 3 == 0 else \
                            nc.gpsimd
                        b2eng.tensor_tensor(
                            P3,
                            u8v[:, (GW // 256) * g:(GW // 256) * (g + 1),
                                :, :, tau % 2],
                            smax3, op=OP.mult)
                    # PV + Z on PE in bf16 (1 cyc/row), tau-major so the
                    # accumulation consumes P/hpb slices as they are built
                    # (overlapping phase 2); two waves of CHG//2 psum tiles
                    PT = Pg[:].rearrange("p (t i) -> p t i", t=NT)
                    HR = hp8[:].rearrange("p (t f) -> p t f", t=NT)
                    for w0 in range(0, CHG, 4):
                        pss = [psum5.tile([128, HPW], dt.float32, tag="pvps",
                                          name=f"pv{g}_{w0}_{i}")
                               for i in range(4)]
                        for u in range(NT):
                            for ic, c in enumerate(range(w0, w0 + 4)):
                                nc.tensor.matmul(
                                    pss[ic][:, 512:F_OUT + 1],
                                    PT[:, u, 128 * c:128 * (c + 1)],
                                    HR[:, u, 512:F_OUT + 1],
                                    start=(u == 0), stop=(u == NT - 1))
                                nc.tensor.matmul(
                                    pss[ic][:, 0:512],
                                    PT[:, u, 128 * c:128 * (c + 1)],
                                    HR[:, u, 0:512],
                                    start=(u == 0), stop=(u == NT - 1))
                        for ic, c in enumerate(range(w0, w0 + 4)):
                            ci = CHG * g + c
                            ps = pss[ic]
                            nc.vector.reciprocal(rz_col[:, ci:ci + 1],
                                                 ps[:, F_OUT:F_OUT + 1])
                            ob = opool.tile([128, F_OUT], dt.float32,
                                            tag="ob")
                            nc.scalar.activation(ob[:], ps[:, 0:F_OUT],
                                                 AF.Copy,
                                                 scale=rz_col[:, ci:ci + 1])
                            if has_bias:
                                nc.gpsimd.tensor_tensor(ob[:], ob[:],
                                                        bias_bc[:],
                                                        op=OP.add)
                            base = sig_off(ci)
                            nc.sync.dma_start(
                                d_out[base:base + 255:2, 0:F_OUT], ob[:])

    nc.compile()
    return nc


def _get_program(has_bias: bool):
    key = ("prog", has_bias)
    if key not in _CACHE:
        _CACHE[key] = _build(has_bias)
    return _CACHE[key]


def kernel(h, adj, w, a_src, a_dst, bias):
    from concourse.bass_utils import run_bass_kernel_spmd

    h = np.ascontiguousarray(np.asarray(h, dtype=np.float32))
    adj_u8 = np.ascontiguousarray(np.asarray(adj).astype(np.uint8))
    w = np.ascontiguousarray(np.asarray(w, dtype=np.float32))
    a_src = np.asarray(a_src, dtype=np.float32).reshape(H, F_OUT)
    a_dst = np.asarray(a_dst, dtype=np.float32).reshape(H, F_OUT)
    bias = np.asarray(bias, dtype=np.float32).reshape(F_OUT)
    has_bias = bool(np.any(bias))

    nc = _get_program(has_bias)

    in_maps = []
    for core in range(NCORES):
        b, hd = core // H, core % H
        m = {
            "h": h[b],
            "adj": adj_u8[b],
            "w": w[hd],
            "a_src": a_src[hd],
            "a_dst": a_dst[hd],
        }
        if has_bias:
            m["bias"] = bias
        in_maps.append(m)

    res = run_bass_kernel_spmd(nc, in_maps, list(range(NCORES)))
    out = np.empty((BS, H, N, F_OUT), dtype=np.float32)
    for core in range(NCORES):
        b, hd = core // H, core % H
        out[b, hd] = res.results[core]["out"]
    return out


# revision 18
# speedup vs baseline: 1.0690x; 1.0262x over previous
"""BatchMultiHeadGraphAttention TRN2 kernel.

Reference computation (per batch b, head h):
    h_prime = h[b] @ w[h]                          # [n, f]
    t = tanh(h_prime)
    src = t @ a_src[h];  dst = t @ a_dst[h]        # [n]
    s[i, j] = leaky_relu(src[i] + dst[j], 0.2)
    s = where(adj[b] | eye, s, -inf)
    attn = softmax(s, axis=-1)
    out[b, h] = attn @ h_prime + bias

Sharding: 8 cores, one (b, h) slab per core (bs=4 x H=2).

Key algebraic restructure vs the naive form: the pre-activation score is
rank-1 (s = src_i + dst_j), so
    exp(lrelu(s)) = max(exp(s), exp(0.2 s)) = v_j * max(u_i, p_i * r_j)
with u=exp(src), v=exp(dst), p=exp(0.2 src), r=exp(-0.8 dst).  The v_j
factor is folded into the value matrix (hp8 = fp8(v_j/4 * h_prime), with
a v_j/4 column computing Z), so the full n x n probability tensor is
built with just TWO elementwise passes (one stt max, one masked multiply)
and ZERO activation-table passes over n^2.  P is stored in fp8e4 and the
PV matmul runs in DoubleRow perf mode (0.5 PE cycles/row).

All n-indices on chip live in the permuted order sigma induced by the
u16-pair xbar transpose of adj (sigma(128*tau + p) = 256*(tau//2) + 2p +
tau%2), applied consistently to j (partitions) and i (free dim), so the
diagonal stays the diagonal and only the final output DMA un-permutes.
"""

import numpy as np

BS, N, H, F_IN, F_OUT = 4, 2048, 2, 768, 768
NCORES = 8

_CACHE = {}


def _build(has_bias: bool):
    import os
    import concourse.bass as bass
    import concourse.mybir as mybir
    import concourse.tile as tile
    from concourse import bacc
    from concourse.masks import make_identity

    dt = mybir.dt
    AF = mybir.ActivationFunctionType
    OP = mybir.AluOpType
    PM = mybir.MatmulPerfMode

    NT = N // 128            # 16 n-tiles (tau)
    KT = F_IN // 128         # 6 k-tiles
    NG = 2                   # i groups
    GW = N // NG             # 1024 group width
    CHG = GW // 128          # 8 i-chunks per group
    HPW = 772                # hp8 row stride (768 + Z col + pad)
    LN4 = float(np.log(4.0))

    nc = bacc.Bacc("TRN2", target_bir_lowering=False, debug=False,
                   num_devices=NCORES)

    d_h = nc.dram_tensor("h", [N, F_IN], dt.float32, kind="ExternalInput")
    d_adj = nc.dram_tensor("adj", [N, N], dt.uint8, kind="ExternalInput")
    d_w = nc.dram_tensor("w", [F_IN, F_OUT], dt.float32, kind="ExternalInput")
    d_asrc = nc.dram_tensor("a_src", [F_OUT], dt.float32, kind="ExternalInput")
    d_adst = nc.dram_tensor("a_dst", [F_OUT], dt.float32, kind="ExternalInput")
    if has_bias:
        d_bias = nc.dram_tensor("bias", [F_OUT], dt.float32,
                                kind="ExternalInput")
    d_out = nc.dram_tensor("out", [N, F_OUT], dt.float32,
                           kind="ExternalOutput")

    def sig_off(tau):
        # on-chip position m = 128*tau + p  <->  logical n index
        # sigma(m) = 256*(tau//2) + 2*p + (tau % 2)
        return 256 * (tau // 2) + (tau % 2)

    with tile.TileContext(nc) as tc:
        with tc.tile_pool(name="const", bufs=1) as cpool, \
             tc.tile_pool(name="persist", bufs=1) as pp:
            # ---- constants ----
            ident = cpool.tile([128, 128], dt.float32, tag="ident")
            make_identity(nc, ident[:])
            eye_u8 = cpool.tile([128, 128], dt.uint8, tag="eye_u8")
            nc.vector.tensor_copy(eye_u8[:], ident[:])
            c_nln4 = cpool.tile([128, 1], dt.float32, tag="c_nln4")
            nc.gpsimd.memset(c_nln4[:], -LN4)

            if has_bias:
                bias_row = cpool.tile([1, F_OUT], dt.float32, tag="bias_row")
                nc.sync.dma_start(bias_row[:],
                                  d_bias.ap().rearrange("(o f) -> o f", o=1))
                bias_bc = pp.tile([128, F_OUT], dt.float32, tag="bias_bc")
                nc.gpsimd.partition_broadcast(bias_bc[:], bias_row[:])

            # ---- persistent buffers ----
            # hpb[j, tau*HPW + f] = bf16(v_j/4 * h_prime[j, f]); col 768 = v_j/4
            hp8 = pp.tile([128, NT * HPW], dt.bfloat16, tag="hp8")
            src_col = pp.tile([128, NT], dt.float32, tag="src_col")
            dst_col = pp.tile([128, NT], dt.float32, tag="dst_col")
            v4_col = pp.tile([128, NT], dt.float32, tag="v4_col")
            r_col = pp.tile([128, NT], dt.float32, tag="r_col")
            rz_col = pp.tile([128, NT], dt.float32, tag="rz_col")

            hp8v = hp8[:].rearrange("p (t f) -> p t f", t=NT)

            # adj staging: 8 tiles [128, N] u16 (xbar transpose of u16 pairs)
            stag = [pp.tile([128, N], dt.uint16, tag=f"stag{t}",
                            name=f"stag{t}")
                    for t in range(NT // 2)]
            adj16 = d_adj.ap().bitcast(dt.uint16)       # [N, N//2]

            # ---- phase 1: hT = transpose(h) via PE; load w ----
            with tc.tile_pool(name="ph1", bufs=6) as hpool, \
                 tc.tile_pool(name="ph1t", bufs=1) as htp, \
                 tc.tile_pool(name="ph1ps", bufs=2, space="PSUM") as psum1:
                # first h group loads ahead of w so PE starts sooner
                hr0 = []
                for nn in range(4):
                    ht_in = hpool.tile([128, F_IN], dt.float32, tag="hin",
                                       name=f"h0_{nn}")
                    nc.sync.dma_start(ht_in[:], d_h[128 * nn:128 * (nn + 1), :])
                    hr0.append(ht_in)
                # w as bf16 (DMA f32 then cast on gpsimd)
                wr = [htp.tile([128, F_OUT], dt.bfloat16, tag=f"wr{k}",
                               name=f"wr{k}")
                      for k in range(KT)]
                for k in range(KT):
                    wtmp = hpool.tile([128, F_OUT], dt.float32, tag="hin",
                                      name=f"wtmp{k}")
                    nc.sync.dma_start(wtmp[:], d_w[128 * k:128 * (k + 1), :])
                    nc.gpsimd.tensor_copy(wr[k][:], wtmp[:])
                # a_src/a_dst broadcast [128, F_OUT] (for the dots)
                asrc_row = htp.tile([1, F_OUT], dt.float32, tag="asrc_row")
                nc.sync.dma_start(asrc_row[:],
                                  d_asrc.ap().rearrange("(o f) -> o f", o=1))
                adst_row = htp.tile([1, F_OUT], dt.float32, tag="adst_row")
                nc.sync.dma_start(adst_row[:],
                                  d_adst.ap().rearrange("(o f) -> o f", o=1))
                asrc_bc = htp.tile([128, F_OUT], dt.float32, tag="asrc_bc")
                nc.gpsimd.partition_broadcast(asrc_bc[:], asrc_row[:])
                adst_bc = htp.tile([128, F_OUT], dt.float32, tag="adst_bc")
                nc.gpsimd.partition_broadcast(adst_bc[:], adst_row[:])

                hT = [htp.tile([128, N], dt.bfloat16, tag=f"hT{k}",
                               name=f"hT{k}")
                      for k in range(KT)]
                for ngrp in range(NT // 4):
                    if ngrp == 0:
                        hr = hr0
                    else:
                        hr = []
                        for nn in range(4):
                            t = 4 * ngrp + nn
                            ht_in = hpool.tile([128, F_IN], dt.float32,
                                               tag="hin")
                            nc.sync.dma_start(ht_in[:],
                                              d_h[128 * t:128 * (t + 1), :])
                            hr.append(ht_in)
                    for k in range(KT):
                        ps = psum1.tile([128, 512], dt.float32, tag="tps")
                        for nn in range(4):
                            nc.tensor.transpose(
                                ps[:, 128 * nn:128 * (nn + 1)],
                                hr[nn][:, 128 * k:128 * (k + 1)],
                                ident[:])
                        # store hT in sigma-permuted column order:
                        # in pos 256*t + 2*q + o -> out pos 256*t+128*o+q
                        psperm = ps[:].rearrange(
                            "p (t q o) -> p t o q", t=2, q=128, o=2)
                        # gpsimd cannot access PSUM; ACT has headroom
                        nc.scalar.copy(
                            hT[k][:, 512 * ngrp:512 * (ngrp + 1)], psperm)

                # adj xbar transposes issued after the h/w loads so they
                # don't hog the HWDGE rings at kernel start
                for t in range(NT // 2):
                    nc.sync.dma_start(stag[t][:],
                                      adj16[:, 128 * t:128 * (t + 1)],
                                      transpose=True)
                # OR the identity into the adj staging at the 16 diagonal
                # blocks: within stag[tau//2], partition q, byte
                # 512*(tau//2) + 4*k + 3*(tau%2) holds adj[logical_i(k),
                # logical_j(q)] for the tau diag block; k==q is the diagonal.
                for tau in range(NT):
                    off = 512 * (tau // 2) + 3 * (tau % 2)
                    dview = stag[tau // 2][:].bitcast(dt.uint8)[
                        :, off:off + 509:4]
                    nc.vector.tensor_tensor(dview, dview, eye_u8[:],
                                            op=OP.max)

                # ---- phase 2: h_prime per tau; tanh; dots; fp8 evict ----
                with tc.tile_pool(name="ph2", bufs=3) as tpool, \
                     tc.tile_pool(name="ph2ps", bufs=3, space="PSUM") as psum2:
                    for tau in range(NT):
                        ps = psum2.tile([128, F_OUT], dt.float32, tag="hpps")
                        for k in range(KT):
                            lhsT = hT[k][:, 128 * tau:128 * (tau + 1)]
                            nc.tensor.matmul(ps[:, 0:512], lhsT,
                                             wr[k][:, 0:512],
                                             start=(k == 0), stop=(k == KT - 1))
                            nc.tensor.matmul(ps[:, 512:F_OUT], lhsT,
                                             wr[k][:, 512:F_OUT],
                                             start=(k == 0), stop=(k == KT - 1))
                        # unscaled fp8 evict (the v_j/4 scale is applied
                        # in-place later, off the psum critical path)
                        nc.scalar.activation(
                            hp8[:, HPW * tau:HPW * tau + F_OUT],
                            ps[:], AF.Copy)
                        # tanh + the two dots
                        tnh = tpool.tile([128, F_OUT], dt.float32, tag="tnh")
                        nc.scalar.activation(tnh[:], ps[:], AF.Tanh)
                        scr = tpool.tile([128, F_OUT], dt.float32, tag="scr")
                        nc.vector.tensor_tensor_reduce(
                            scr[:], tnh[:], asrc_bc[:], 1.0, 0.0,
                            op0=OP.mult, op1=OP.add,
                            accum_out=src_col[:, tau:tau + 1])
                        scr2 = tpool.tile([128, F_OUT], dt.float32, tag="scr")
                        nc.vector.tensor_tensor_reduce(
                            scr2[:], tnh[:], adst_bc[:], 1.0, 0.0,
                            op0=OP.mult, op1=OP.add,
                            accum_out=dst_col[:, tau:tau + 1])
                        if tau % 4 == 3:
                            # batched j-side factors for taus [tau-3, tau]:
                            # v/4 = exp(dst - ln4), r = exp(-0.8 dst), then
                            # scale hpb rows by v/4 (Pool) + write Z cols
                            q0 = tau - 3
                            nc.scalar.activation(v4_col[:, q0:tau + 1],
                                                 dst_col[:, q0:tau + 1],
                                                 AF.Exp, bias=c_nln4[:])
                            nc.scalar.activation(r_col[:, q0:tau + 1],
                                                 dst_col[:, q0:tau + 1],
                                                 AF.Exp, scale=-0.8)
                            nc.vector.tensor_copy(
                                hp8v[:, q0:tau + 1, F_OUT],
                                v4_col[:, q0:tau + 1])
                            for q in range(q0, tau + 1):
                                nc.gpsimd.tensor_scalar(
                                    hp8[:, HPW * q:HPW * q + F_OUT],
                                    hp8[:, HPW * q:HPW * q + F_OUT],
                                    v4_col[:, q:q + 1], None, op0=OP.mult)

            # ---- phase 3+4, per i-group g (= tau half-range) ----
            # group g's u/p factors need only src from taus [8g, 8g+8), so
            # g=0's P build starts while phase 2 finishes taus 8..15.
            HT = NT // NG  # taus per group
            u_bc = pp.tile([128, N], dt.float32, tag="u_bc")
            p_bc = pp.tile([128, N], dt.float32, tag="p_bc")
            with tc.tile_pool(name="ph3", bufs=2) as p3, \
                 tc.tile_pool(name="pg", bufs=4) as sp, \
                 tc.tile_pool(name="pgP", bufs=1) as ppool, \
                 tc.tile_pool(name="pgo", bufs=4) as opool, \
                 tc.tile_pool(name="pgps", bufs=4, space="PSUM") as psum5:
                for g in range(NG):
                    t0 = HT * g
                    # i-side factors for this group: exp then gather-DMA
                    # (per-element descriptors) into a [1, GW] row, then
                    # partition-broadcast
                    ux = p3.tile([128, HT], dt.float32, tag="ux")
                    nc.scalar.activation(ux[:], src_col[:, t0:t0 + HT],
                                         AF.Exp)
                    px = p3.tile([128, HT], dt.float32, tag="px")
                    nc.scalar.activation(px[:], src_col[:, t0:t0 + HT],
                                         AF.Exp, scale=0.2)
                    u_row = p3.tile([1, GW], dt.float32, tag="u_row")
                    nc.sync.dma_start(
                        u_row[:].rearrange("o (t p) -> o t p", t=HT),
                        ux[:].rearrange("p t -> t p"))
                    p_row = p3.tile([1, GW], dt.float32, tag="p_row")
                    nc.sync.dma_start(
                        p_row[:].rearrange("o (t p) -> o t p", t=HT),
                        px[:].rearrange("p t -> t p"))
                    nc.gpsimd.partition_broadcast(
                        u_bc[:, GW * g:GW * (g + 1)], u_row[:])
                    nc.gpsimd.partition_broadcast(
                        p_bc[:, GW * g:GW * (g + 1)], p_row[:])

                    Pg = ppool.tile([128, NT * GW], dt.bfloat16, tag=f"P{g}",
                                    name=f"P{g}")
                    for tau in range(NT):
                        # smax = max(u_i, p_i * r_j)
                        smax = sp.tile([128, GW], dt.float32, tag="smax")
                        nc.vector.scalar_tensor_tensor(
                            smax[:], p_bc[:, GW * g:GW * (g + 1)],
                            r_col[:, tau:tau + 1],
                            u_bc[:, GW * g:GW * (g + 1)],
                            op0=OP.mult, op1=OP.max)
                        # P = mask * smax (fp8), mask via the u8 adj view
                        u8v = stag[tau // 2][:].bitcast(dt.uint8).rearrange(
                            "p (b k hh o) -> p b hh k o",
                            b=N // 256, k=128, hh=2, o=2)
                        smax3 = smax[:].rearrange(
                            "p (b hh k) -> p b hh k", b=GW // 256, hh=2, k=128)
                        P3 = Pg[:, GW * tau:GW * (tau + 1)].rearrange(
                            "p (b hh k) -> p b hh k", b=GW // 256, hh=2, k=128)
                        # mask multiply: mostly on Pool (it only supports
                        # mult), the rest on DVE for balance
                        b2eng = nc.vector if (NT * g + tau) % 3 == 0 else \
                            nc.gpsimd
                        b2eng.tensor_tensor(
                            P3,
                            u8v[:, (GW // 256) * g:(GW // 256) * (g + 1),
                                :, :, tau % 2],
                            smax3, op=OP.mult)
                    # PV + Z on PE in bf16 (1 cyc/row), tau-major so the
                    # accumulation consumes P/hpb slices as they are built
                    # (overlapping phase 2); two waves of CHG//2 psum tiles
                    PT = Pg[:].rearrange("p (t i) -> p t i", t=NT)
                    HR = hp8[:].rearrange("p (t f) -> p t f", t=NT)
                    for w0 in range(0, CHG, 4):
                        pss = [psum5.tile([128, HPW], dt.float32, tag="pvps",
                                          name=f"pv{g}_{w0}_{i}")
                               for i in range(4)]
                        for u in range(NT):
                            for ic, c in enumerate(range(w0, w0 + 4)):
                                nc.tensor.matmul(
                                    pss[ic][:, 512:F_OUT + 1],
                                    PT[:, u, 128 * c:128 * (c + 1)],
                                    HR[:, u, 512:F_OUT + 1],
                                    start=(u == 0), stop=(u == NT - 1))
                                nc.tensor.matmul(
                                    pss[ic][:, 0:512],
                                    PT[:, u, 128 * c:128 * (c + 1)],
                                    HR[:, u, 0:512],
                                    start=(u == 0), stop=(u == NT - 1))
                        for ic, c in enumerate(range(w0, w0 + 4)):
                            ci = CHG * g + c
                            ps = pss[ic]
                            nc.vector.reciprocal(rz_col[:, ci:ci + 1],
                                                 ps[:, F_OUT:F_OUT + 1])
                            ob = opool.tile([128, F_OUT], dt.float32,
                                            tag="ob")
                            nc.scalar.activation(ob[:], ps[:, 0:F_OUT],
                                                 AF.Copy,
                                                 scale=rz_col[:, ci:ci + 1])
                            if has_bias:
                                nc.gpsimd.tensor_tensor(ob[:], ob[:],
                                                        bias_bc[:],
                                                        op=OP.add)
                            base = sig_off(ci)
                            nc.sync.dma_start(
                                d_out[base:base + 255:2, 0:F_OUT], ob[:])

    nc.compile()
    return nc


def _get_program(has_bias: bool):
    key = ("prog", has_bias)
    if key not in _CACHE:
        _CACHE[key] = _build(has_bias)
    return _CACHE[key]


def kernel(h, adj, w, a_src, a_dst, bias):
    from concourse.bass_utils import run_bass_kernel_spmd

    h = np.ascontiguousarray(np.asarray(h, dtype=np.float32))
    adj_u8 = np.ascontiguousarray(np.asarray(adj).astype(np.uint8))
    w = np.ascontiguousarray(np.asarray(w, dtype=np.float32))
    a_src = np.asarray(a_src, dtype=np.float32).reshape(H, F_OUT)
    a_dst = np.asarray(a_dst, dtype=np.float32).reshape(H, F_OUT)
    bias = np.asarray(bias, dtype=np.float32).reshape(F_OUT)
    has_bias = bool(np.any(bias))

    nc = _get_program(has_bias)

    in_maps = []
    for core in range(NCORES):
        b, hd = core // H, core % H
        m = {
            "h": h[b],
            "adj": adj_u8[b],
            "w": w[hd],
            "a_src": a_src[hd],
            "a_dst": a_dst[hd],
        }
        if has_bias:
            m["bias"] = bias
        in_maps.append(m)

    res = run_bass_kernel_spmd(nc, in_maps, list(range(NCORES)))
    out = np.empty((BS, H, N, F_OUT), dtype=np.float32)
    for core in range(NCORES):
        b, hd = core // H, core % H
        out[b, hd] = res.results[core]["out"]
    return out


# revision 19
# speedup vs baseline: 1.0952x; 1.0245x over previous
"""BatchMultiHeadGraphAttention TRN2 kernel.

Reference computation (per batch b, head h):
    h_prime = h[b] @ w[h]                          # [n, f]
    t = tanh(h_prime)
    src = t @ a_src[h];  dst = t @ a_dst[h]        # [n]
    s[i, j] = leaky_relu(src[i] + dst[j], 0.2)
    s = where(adj[b] | eye, s, -inf)
    attn = softmax(s, axis=-1)
    out[b, h] = attn @ h_prime + bias

Sharding: 8 cores, one (b, h) slab per core (bs=4 x H=2).

Key algebraic restructure vs the naive form: the pre-activation score is
rank-1 (s = src_i + dst_j), so
    exp(lrelu(s)) = max(exp(s), exp(0.2 s)) = v_j * max(u_i, p_i * r_j)
with u=exp(src), v=exp(dst), p=exp(0.2 src), r=exp(-0.8 dst).  The v_j
factor is folded into the value matrix (hp8 = fp8(v_j/4 * h_prime), with
a v_j/4 column computing Z), so the full n x n probability tensor is
built with just TWO elementwise passes (one stt max, one masked multiply)
and ZERO activation-table passes over n^2.  P is stored in fp8e4 and the
PV matmul runs in DoubleRow perf mode (0.5 PE cycles/row).

All n-indices on chip live in the permuted order sigma induced by the
u16-pair xbar transpose of adj (sigma(128*tau + p) = 256*(tau//2) + 2p +
tau%2), applied consistently to j (partitions) and i (free dim), so the
diagonal stays the diagonal and only the final output DMA un-permutes.
"""

import numpy as np

BS, N, H, F_IN, F_OUT = 4, 2048, 2, 768, 768
NCORES = 8

_CACHE = {}


def _build(has_bias: bool):
    import os
    import concourse.bass as bass
    import concourse.mybir as mybir
    import concourse.tile as tile
    from concourse import bacc
    from concourse.masks import make_identity

    dt = mybir.dt
    AF = mybir.ActivationFunctionType
    OP = mybir.AluOpType
    PM = mybir.MatmulPerfMode

    NT = N // 128            # 16 n-tiles (tau)
    KT = F_IN // 128         # 6 k-tiles
    NG = 2                   # i groups
    GW = N // NG             # 1024 group width
    CHG = GW // 128          # 8 i-chunks per group
    HPW = 772                # hp8 row stride (768 + Z col + pad)
    LN4 = float(np.log(4.0))

    nc = bacc.Bacc("TRN2", target_bir_lowering=False, debug=False,
                   num_devices=NCORES)

    d_h = nc.dram_tensor("h", [N, F_IN], dt.float32, kind="ExternalInput")
    d_adj = nc.dram_tensor("adj", [N, N], dt.uint8, kind="ExternalInput")
    d_w = nc.dram_tensor("w", [F_IN, F_OUT], dt.float32, kind="ExternalInput")
    d_asrc = nc.dram_tensor("a_src", [F_OUT], dt.float32, kind="ExternalInput")
    d_adst = nc.dram_tensor("a_dst", [F_OUT], dt.float32, kind="ExternalInput")
    if has_bias:
        d_bias = nc.dram_tensor("bias", [F_OUT], dt.float32,
                                kind="ExternalInput")
    d_out = nc.dram_tensor("out", [N, F_OUT], dt.float32,
                           kind="ExternalOutput")

    def sig_off(tau):
        # on-chip position m = 128*tau + p  <->  logical n index
        # sigma(m) = 256*(tau//2) + 2*p + (tau % 2)
        return 256 * (tau // 2) + (tau % 2)

    with tile.TileContext(nc) as tc:
        with tc.tile_pool(name="const", bufs=1) as cpool, \
             tc.tile_pool(name="persist", bufs=1) as pp:
            # ---- constants ----
            ident = cpool.tile([128, 128], dt.float32, tag="ident")
            make_identity(nc, ident[:])
            eye_u8 = cpool.tile([128, 128], dt.uint8, tag="eye_u8")
            nc.vector.tensor_copy(eye_u8[:], ident[:])
            c_nln4 = cpool.tile([128, 1], dt.float32, tag="c_nln4")
            nc.gpsimd.memset(c_nln4[:], -LN4)

            if has_bias:
                bias_row = cpool.tile([1, F_OUT], dt.float32, tag="bias_row")
                nc.sync.dma_start(bias_row[:],
                                  d_bias.ap().rearrange("(o f) -> o f", o=1))
                bias_bc = pp.tile([128, F_OUT], dt.float32, tag="bias_bc")
                nc.gpsimd.partition_broadcast(bias_bc[:], bias_row[:])

            # ---- persistent buffers ----
            # hpb[j, tau*HPW + f] = bf16(v_j/4 * h_prime[j, f]); col 768 = v_j/4
            hp8 = pp.tile([128, NT * HPW], dt.bfloat16, tag="hp8")
            src_col = pp.tile([128, NT], dt.float32, tag="src_col")
            dst_col = pp.tile([128, NT], dt.float32, tag="dst_col")
            v4_col = pp.tile([128, NT], dt.float32, tag="v4_col")
            r_col = pp.tile([128, NT], dt.float32, tag="r_col")
            rz_col = pp.tile([128, NT], dt.float32, tag="rz_col")

            hp8v = hp8[:].rearrange("p (t f) -> p t f", t=NT)

            # adj staging: 8 tiles [128, N] u16 (xbar transpose of u16 pairs)
            stag = [pp.tile([128, N], dt.uint16, tag=f"stag{t}",
                            name=f"stag{t}")
                    for t in range(NT // 2)]
            adj16 = d_adj.ap().bitcast(dt.uint16)       # [N, N//2]

            # ---- phase 1: hT = transpose(h) via PE; load w ----
            with tc.tile_pool(name="ph1", bufs=6) as hpool, \
                 tc.tile_pool(name="ph1t", bufs=1) as htp, \
                 tc.tile_pool(name="ph1ps", bufs=2, space="PSUM") as psum1:
                # first h group loads ahead of w so PE starts sooner
                hr0 = []
                for nn in range(4):
                    ht_in = hpool.tile([128, F_IN], dt.float32, tag="hin",
                                       name=f"h0_{nn}")
                    nc.sync.dma_start(ht_in[:], d_h[128 * nn:128 * (nn + 1), :])
                    hr0.append(ht_in)
                # w as bf16 (DMA f32 then cast on gpsimd)
                wr = [htp.tile([128, F_OUT], dt.bfloat16, tag=f"wr{k}",
                               name=f"wr{k}")
                      for k in range(KT)]
                for k in range(KT):
                    wtmp = hpool.tile([128, F_OUT], dt.float32, tag="hin",
                                      name=f"wtmp{k}")
                    nc.sync.dma_start(wtmp[:], d_w[128 * k:128 * (k + 1), :])
                    nc.gpsimd.tensor_copy(wr[k][:], wtmp[:])
                # a_src/a_dst broadcast [128, F_OUT] (for the dots)
                asrc_row = htp.tile([1, F_OUT], dt.float32, tag="asrc_row")
                nc.sync.dma_start(asrc_row[:],
                                  d_asrc.ap().rearrange("(o f) -> o f", o=1))
                adst_row = htp.tile([1, F_OUT], dt.float32, tag="adst_row")
                nc.sync.dma_start(adst_row[:],
                                  d_adst.ap().rearrange("(o f) -> o f", o=1))
                asrc_bc = htp.tile([128, F_OUT], dt.float32, tag="asrc_bc")
                nc.gpsimd.partition_broadcast(asrc_bc[:], asrc_row[:])
                adst_bc = htp.tile([128, F_OUT], dt.float32, tag="adst_bc")
                nc.gpsimd.partition_broadcast(adst_bc[:], adst_row[:])

                hT = [htp.tile([128, N], dt.bfloat16, tag=f"hT{k}",
                               name=f"hT{k}")
                      for k in range(KT)]
                for ngrp in range(NT // 4):
                    if ngrp == 0:
                        hr = hr0
                    else:
                        hr = []
                        for nn in range(4):
                            t = 4 * ngrp + nn
                            ht_in = hpool.tile([128, F_IN], dt.float32,
                                               tag="hin")
                            nc.sync.dma_start(ht_in[:],
                                              d_h[128 * t:128 * (t + 1), :])
                            hr.append(ht_in)
                    for k in range(KT):
                        ps = psum1.tile([128, 512], dt.float32, tag="tps")
                        for nn in range(4):
                            nc.tensor.transpose(
                                ps[:, 128 * nn:128 * (nn + 1)],
                                hr[nn][:, 128 * k:128 * (k + 1)],
                                ident[:])
                        # store hT in sigma-permuted column order:
                        # in pos 256*t + 2*q + o -> out pos 256*t+128*o+q
                        psperm = ps[:].rearrange(
                            "p (t q o) -> p t o q", t=2, q=128, o=2)
                        # gpsimd cannot access PSUM; ACT has headroom
                        nc.scalar.copy(
                            hT[k][:, 512 * ngrp:512 * (ngrp + 1)], psperm)

                # adj xbar transposes issued after the h/w loads so they
                # don't hog the HWDGE rings at kernel start
                for t in range(NT // 2):
                    nc.sync.dma_start(stag[t][:],
                                      adj16[:, 128 * t:128 * (t + 1)],
                                      transpose=True)
                # OR the identity into the adj staging at the 16 diagonal
                # blocks: within stag[tau//2], partition q, byte
                # 512*(tau//2) + 4*k + 3*(tau%2) holds adj[logical_i(k),
                # logical_j(q)] for the tau diag block; k==q is the diagonal.
                for tau in range(NT):
                    off = 512 * (tau // 2) + 3 * (tau % 2)
                    dview = stag[tau // 2][:].bitcast(dt.uint8)[
                        :, off:off + 509:4]
                    nc.vector.tensor_tensor(dview, dview, eye_u8[:],
                                            op=OP.max)

                # ---- phase 2: h_prime per tau; tanh; dots; fp8 evict ----
                with tc.tile_pool(name="ph2", bufs=3) as tpool, \
                     tc.tile_pool(name="ph2ps", bufs=3, space="PSUM") as psum2:
                    for tau in range(NT):
                        ps = psum2.tile([128, F_OUT], dt.float32, tag="hpps")
                        for k in range(KT):
                            lhsT = hT[k][:, 128 * tau:128 * (tau + 1)]
                            nc.tensor.matmul(ps[:, 0:512], lhsT,
                                             wr[k][:, 0:512],
                                             start=(k == 0), stop=(k == KT - 1))
                            nc.tensor.matmul(ps[:, 512:F_OUT], lhsT,
                                             wr[k][:, 512:F_OUT],
                                             start=(k == 0), stop=(k == KT - 1))
                        # tanh + the two dots
                        tnh = tpool.tile([128, F_OUT], dt.float32, tag="tnh")
                        nc.scalar.activation(tnh[:], ps[:], AF.Tanh)
                        scr = tpool.tile([128, F_OUT], dt.float32, tag="scr")
                        nc.vector.tensor_tensor_reduce(
                            scr[:], tnh[:], asrc_bc[:], 1.0, 0.0,
                            op0=OP.mult, op1=OP.add,
                            accum_out=src_col[:, tau:tau + 1])
                        scr2 = tpool.tile([128, F_OUT], dt.float32, tag="scr")
                        nc.vector.tensor_tensor_reduce(
                            scr2[:], tnh[:], adst_bc[:], 1.0, 0.0,
                            op0=OP.mult, op1=OP.add,
                            accum_out=dst_col[:, tau:tau + 1])
                        # per-tau j-side factors, then the scaled evict
                        # hpb = bf16(v_j/4 * h_prime) straight from psum
                        nc.scalar.activation(v4_col[:, tau:tau + 1],
                                             dst_col[:, tau:tau + 1],
                                             AF.Exp, bias=c_nln4[:])
                        nc.scalar.activation(r_col[:, tau:tau + 1],
                                             dst_col[:, tau:tau + 1],
                                             AF.Exp, scale=-0.8)
                        nc.scalar.activation(
                            hp8[:, HPW * tau:HPW * tau + F_OUT],
                            ps[:], AF.Copy, scale=v4_col[:, tau:tau + 1])
                        if tau % 4 == 3:
                            nc.vector.tensor_copy(
                                hp8v[:, tau - 3:tau + 1, F_OUT],
                                v4_col[:, tau - 3:tau + 1])

            # ---- phase 3+4, per i-group g (= tau half-range) ----
            # group g's u/p factors need only src from taus [8g, 8g+8), so
            # g=0's P build starts while phase 2 finishes taus 8..15.
            HT = NT // NG  # taus per group
            u_bc = pp.tile([128, N], dt.float32, tag="u_bc")
            p_bc = pp.tile([128, N], dt.float32, tag="p_bc")
            with tc.tile_pool(name="ph3", bufs=2) as p3, \
                 tc.tile_pool(name="pg", bufs=4) as sp, \
                 tc.tile_pool(name="pgP", bufs=1) as ppool, \
                 tc.tile_pool(name="pgo", bufs=4) as opool, \
                 tc.tile_pool(name="pgps", bufs=4, space="PSUM") as psum5:
                for g in range(NG):
                    t0 = HT * g
                    # i-side factors for this group: exp then gather-DMA
                    # (per-element descriptors) into a [1, GW] row, then
                    # partition-broadcast
                    ux = p3.tile([128, HT], dt.float32, tag="ux")
                    nc.scalar.activation(ux[:], src_col[:, t0:t0 + HT],
                                         AF.Exp)
                    px = p3.tile([128, HT], dt.float32, tag="px")
                    nc.scalar.activation(px[:], src_col[:, t0:t0 + HT],
                                         AF.Exp, scale=0.2)
                    u_row = p3.tile([1, GW], dt.float32, tag="u_row")
                    nc.sync.dma_start(
                        u_row[:].rearrange("o (t p) -> o t p", t=HT),
                        ux[:].rearrange("p t -> t p"))
                    p_row = p3.tile([1, GW], dt.float32, tag="p_row")
                    nc.sync.dma_start(
                        p_row[:].rearrange("o (t p) -> o t p", t=HT),
                        px[:].rearrange("p t -> t p"))
                    nc.gpsimd.partition_broadcast(
                        u_bc[:, GW * g:GW * (g + 1)], u_row[:])
                    nc.gpsimd.partition_broadcast(
                        p_bc[:, GW * g:GW * (g + 1)], p_row[:])

                    Pg = ppool.tile([128, NT * GW], dt.bfloat16, tag=f"P{g}",
                                    name=f"P{g}")
                    for tau in range(NT):
                        # smax = max(u_i, p_i * r_j)
                        smax = sp.tile([128, GW], dt.float32, tag="smax")
                        nc.vector.scalar_tensor_tensor(
                            smax[:], p_bc[:, GW * g:GW * (g + 1)],
                            r_col[:, tau:tau + 1],
                            u_bc[:, GW * g:GW * (g + 1)],
                            op0=OP.mult, op1=OP.max)
                        # P = mask * smax (fp8), mask via the u8 adj view
                        u8v = stag[tau // 2][:].bitcast(dt.uint8).rearrange(
                            "p (b k hh o) -> p b hh k o",
                            b=N // 256, k=128, hh=2, o=2)
                        smax3 = smax[:].rearrange(
                            "p (b hh k) -> p b hh k", b=GW // 256, hh=2, k=128)
                        P3 = Pg[:, GW * tau:GW * (tau + 1)].rearrange(
                            "p (b hh k) -> p b hh k", b=GW // 256, hh=2, k=128)
                        # mask multiply: mostly on Pool (it only supports
                        # mult), the rest on DVE for balance
                        b2eng = nc.vector if (NT * g + tau) % 3 == 0 else \
                            nc.gpsimd
                        b2eng.tensor_tensor(
                            P3,
                            u8v[:, (GW // 256) * g:(GW // 256) * (g + 1),
                                :, :, tau % 2],
                            smax3, op=OP.mult)
                    # PV + Z on PE in bf16 (1 cyc/row), tau-major so the
                    # accumulation consumes P/hpb slices as they are built
                    # (overlapping phase 2); two waves of CHG//2 psum tiles
                    PT = Pg[:].rearrange("p (t i) -> p t i", t=NT)
                    HR = hp8[:].rearrange("p (t f) -> p t f", t=NT)
                    for w0 in range(0, CHG, 4):
                        pss = [psum5.tile([128, HPW], dt.float32, tag="pvps",
                                          name=f"pv{g}_{w0}_{i}")
                               for i in range(4)]
                        for u in range(NT):
                            for ic, c in enumerate(range(w0, w0 + 4)):
                                nc.tensor.matmul(
                                    pss[ic][:, 512:F_OUT + 1],
                                    PT[:, u, 128 * c:128 * (c + 1)],
                                    HR[:, u, 512:F_OUT + 1],
                                    start=(u == 0), stop=(u == NT - 1))
                                nc.tensor.matmul(
                                    pss[ic][:, 0:512],
                                    PT[:, u, 128 * c:128 * (c + 1)],
                                    HR[:, u, 0:512],
                                    start=(u == 0), stop=(u == NT - 1))
                        for ic, c in enumerate(range(w0, w0 + 4)):
                            ci = CHG * g + c
                            ps = pss[ic]
                            nc.vector.reciprocal(rz_col[:, ci:ci + 1],
                                                 ps[:, F_OUT:F_OUT + 1])
                            ob = opool.tile([128, F_OUT], dt.float32,
                                            tag="ob")
                            nc.scalar.activation(ob[:], ps[:, 0:F_OUT],
                                                 AF.Copy,
                                                 scale=rz_col[:, ci:ci + 1])
                            if has_bias:
                                nc.gpsimd.tensor_tensor(ob[:], ob[:],
                                                        bias_bc[:],
                                                        op=OP.add)
                            base = sig_off(ci)
                            nc.sync.dma_start(
                                d_out[base:base + 255:2, 0:F_OUT], ob[:])

    nc.compile()
    return nc


def _get_program(has_bias: bool):
    key = ("prog", has_bias)
    if key not in _CACHE:
        _CACHE[key] = _build(has_bias)
    return _CACHE[key]


def kernel(h, adj, w, a_src, a_dst, bias):
    from concourse.bass_utils import run_bass_kernel_spmd

    h = np.ascontiguousarray(np.asarray(h, dtype=np.float32))
    adj_u8 = np.ascontiguousarray(np.asarray(adj).astype(np.uint8))
    w = np.ascontiguousarray(np.asarray(w, dtype=np.float32))
    a_src = np.asarray(a_src, dtype=np.float32).reshape(H, F_OUT)
    a_dst = np.asarray(a_dst, dtype=np.float32).reshape(H, F_OUT)
    bias = np.asarray(bias, dtype=np.float32).reshape(F_OUT)
    has_bias = bool(np.any(bias))

    nc = _get_program(has_bias)

    in_maps = []
    for core in range(NCORES):
        b, hd = core // H, core % H
        m = {
            "h": h[b],
            "adj": adj_u8[b],
            "w": w[hd],
            "a_src": a_src[hd],
            "a_dst": a_dst[hd],
        }
        if has_bias:
            m["bias"] = bias
        in_maps.append(m)

    res = run_bass_kernel_spmd(nc, in_maps, list(range(NCORES)))
    out = np.empty((BS, H, N, F_OUT), dtype=np.float32)
    for core in range(NCORES):
        b, hd = core // H, core % H
        out[b, hd] = res.results[core]["out"]
    return out
